# revision 6
# baseline (speedup 1.0000x reference)
"""Causal self-attention (B=16, S=2048, D=512) on 8 Trainium2 NeuronCores.

Strategy: data-parallel over batch (2 sequences per core), QKV weights
replicated. Per sequence everything is computed in transposed layouts so no
on-device transposes are needed:

  host prep:  xT = x^T per sequence [D, S];  wqT = Wq^T/sqrt(D);  wkT = Wk^T;
              wvT = Wv^T;  bq' = bq/sqrt(D);  key-pad bias (0/-1e30);
              query mask (1/0) as floats.

  device (per sequence):
    QT[d,s]  = wqT^T.slices @ xT        (+bq' via DVE eviction, pre-scaled)
    KT[d,s]  = wkT^T.slices @ xT        (+bk via DVE eviction)
    V[s,d]   = xT^T.slices @ wvT        (+bv via rank-1 ones matmul)
    per q-chunk (512 queries):
      scoresT[k,q] = KT.T @ QT          (causal block-skip; diagonal blocks
                                         get a precomputed triangular -1e30
                                         tile added on DVE)
      expT = Exp(scoresT + keybias[k])  (ACT, key-pad mask via bias; no
                                         max-subtraction needed: |scores|<~40)
      denom[q] = ones.T @ expT          (PE reduction over k)
      out_un[q,d] = expT.slices^T @ V   (accumulate over k blocks <= q block)
      out = out_un * (recip(denom+eps) * qmask)[q]   (per-partition DVE scale)

softmax equivalence: exp without max-subtraction, masked entries exactly 0;
rows with a padded query are zeroed by qmask (matches reference's
post-softmax zeroing); trailing or arbitrary masks are both handled exactly
(an unpadded query q always has key q unmasked, so denom > 0).

All matmuls run in float32r (full-rate fp32 PE mode, ~1e-4 rel err).
"""

import numpy as np

import concourse.bacc as bacc
import concourse.mybir as mybir
from concourse.tile import TileContext
from concourse.bass_utils import run_bass_kernel_spmd

B, S, D = 16, 2048, 512
N_CORES = 8
BPC = B // N_CORES          # sequences per core
P = 128                     # partition dim
W = 512                     # matmul moving width (one PSUM bank of fp32)
DC = D // P                 # 4 contraction chunks of 128 over D
SB = S // P                 # 16 blocks of 128 over S (k/q/s blocks)
QC = S // W                 # 4 query chunks of 512
NEG = -1.0e30
EPS = 1.0e-30

f32 = mybir.dt.float32
f32r = mybir.dt.float32r


def build_nc(repeat: int = 1):
    nc = bacc.Bacc()

    xT_d = nc.declare_dram_parameter("xT", [BPC, D, S], f32r, isOutput=False)
    wqT_d = nc.declare_dram_parameter("wqT", [D, D], f32r, isOutput=False)
    wkT_d = nc.declare_dram_parameter("wkT", [D, D], f32r, isOutput=False)
    wvT_d = nc.declare_dram_parameter("wvT", [D, D], f32r, isOutput=False)
    bq_d = nc.declare_dram_parameter("bq", [D], f32, isOutput=False)
    bk_d = nc.declare_dram_parameter("bk", [D], f32, isOutput=False)
    bv_d = nc.declare_dram_parameter("bv", [1, D], f32r, isOutput=False)
    kbias_d = nc.declare_dram_parameter("kbias", [BPC, S], f32, isOutput=False)
    qmask_d = nc.declare_dram_parameter("qmask", [BPC, S], f32, isOutput=False)
    ones_d = nc.declare_dram_parameter("ones", [P, 1], f32r, isOutput=False)
    onesr_d = nc.declare_dram_parameter("onesr", [1, P], f32r, isOutput=False)
    out_d = nc.declare_dram_parameter("out", [BPC, S, D], f32, isOutput=True)

    with TileContext(nc) as tc:
        with (
            tc.tile_pool(name="persist", bufs=1) as pers,
            tc.tile_pool(name="xt", bufs=DC) as xp,
            tc.tile_pool(name="qt", bufs=DC) as qp,
            tc.tile_pool(name="kt", bufs=DC) as kp,
            tc.tile_pool(name="vv", bufs=SB) as vp,
            tc.tile_pool(name="we", bufs=16) as wep,
            tc.tile_pool(name="outs", bufs=3) as op_,
            tc.tile_pool(name="misc", bufs=4) as mp,
            tc.tile_pool(name="pp", bufs=2, space="PSUM") as pp,
            tc.tile_pool(name="ps", bufs=2, space="PSUM") as psp,
            tc.tile_pool(name="pd", bufs=1, space="PSUM") as pdp,
            tc.tile_pool(name="po", bufs=2, space="PSUM") as pop,
        ):
            # ---- persistent setup (once) ----
            ones_t = pers.tile([P, 1], f32r, tag="ones")
            nc.sync.dma_start(out=ones_t[:], in_=ones_d[:])
            onesr_t = pers.tile([1, P], f32r, tag="onesr")
            nc.sync.dma_start(out=onesr_t[:], in_=onesr_d[:])
            bv_t = pers.tile([1, D], f32r, tag="bv")
            nc.sync.dma_start(out=bv_t[:], in_=bv_d[:])
            bq_t = pers.tile([P, DC], f32, tag="bq")
            nc.sync.dma_start(out=bq_t[:], in_=bq_d.rearrange("(n p) -> p n", p=P))
            bk_t = pers.tile([P, DC], f32, tag="bk")
            nc.sync.dma_start(out=bk_t[:], in_=bk_d.rearrange("(n p) -> p n", p=P))

            # identity for PE-mode transpose of the [4,128] denominator strip
            ident = pers.tile([P, P], f32, tag="ident")
            nc.gpsimd.memset(ident[:], 0.0)
            nc.gpsimd.affine_select(
                out=ident[:], in_=ident[:],
                compare_op=mybir.AluOpType.not_equal, fill=1.0,
                base=0, pattern=[[-1, P]], channel_multiplier=1,
            )

            # causal tiles for the 4 diagonal sub-blocks of a [k=128, q=512]
            # scoresT tile: mask (set NEG) where 128*j + k_local > q_local
            causal = []
            for j in range(W // P):
                ct = pers.tile([P, W], f32, tag=f"causal{j}")
                nc.gpsimd.memset(ct[:], 0.0)
                nc.gpsimd.affine_select(
                    out=ct[:],
                    in_=ct[:],
                    compare_op=mybir.AluOpType.is_ge,
                    fill=NEG,
                    base=-P * j,
                    pattern=[[1, W]],
                    channel_multiplier=-1,
                )
                causal.append(ct)

            for _rep in range(repeat):
                for seq in range(BPC):
                    # ---- per-sequence masks ----
                    kbias_t = mp.tile([P, SB], f32, tag="kbias")
                    nc.sync.dma_start(
                        out=kbias_t[:],
                        in_=kbias_d[seq].rearrange("(n p) -> p n", p=P),
                    )
                    qmask_t = mp.tile([P, SB], f32, tag="qmask")
                    nc.sync.dma_start(
                        out=qmask_t[:],
                        in_=qmask_d[seq].rearrange("(n p) -> p n", p=P),
                    )

                    # ---- load xT (4 tiles of [128, S]) ----
                    xt = []
                    for c in range(DC):
                        t = xp.tile([P, S], f32r, tag="xt")
                        nc.sync.dma_start(
                            out=t[:], in_=xT_d[seq, c * P:(c + 1) * P, :]
                        )
                        xt.append(t)

                    # ---- load weights (shared slots with exp tiles) ----
                    wq, wk, wv = [], [], []
                    for c in range(DC):
                        t = wep.tile([P, W], f32r, tag="we")
                        nc.sync.dma_start(out=t[:], in_=wqT_d[c * P:(c + 1) * P, :])
                        wq.append(t)
                    for c in range(DC):
                        t = wep.tile([P, W], f32r, tag="we")
                        nc.sync.dma_start(out=t[:], in_=wkT_d[c * P:(c + 1) * P, :])
                        wk.append(t)
                    for c in range(DC):
                        t = wep.tile([P, W], f32r, tag="we")
                        nc.sync.dma_start(out=t[:], in_=wvT_d[c * P:(c + 1) * P, :])
                        wv.append(t)

                    # ---- projections ----
                    # QT/KT: [d_block 128, s] accumulated over c, bias via DVE
                    qT, kT = [], []
                    for db in range(DC):
                        tq = qp.tile([P, S], f32r, tag="qt")
                        tk = kp.tile([P, S], f32r, tag="kt")
                        qT.append(tq)
                        kT.append(tk)
                    for db in range(DC):
                        for sc in range(QC):
                            pq = pp.tile([P, W], f32, tag="pp")
                            for c in range(DC):
                                nc.tensor.matmul(
                                    pq[:],
                                    wq[c][:, db * P:(db + 1) * P],
                                    xt[c][:, sc * W:(sc + 1) * W],
                                    start=(c == 0),
                                    stop=(c == DC - 1),
                                )
                            nc.vector.tensor_scalar_add(
                                qT[db][:, sc * W:(sc + 1) * W],
                                pq[:],
                                bq_t[:, db:db + 1],
                            )
                            pk = pp.tile([P, W], f32, tag="pp")
                            for c in range(DC):
                                nc.tensor.matmul(
                                    pk[:],
                                    wk[c][:, db * P:(db + 1) * P],
                                    xt[c][:, sc * W:(sc + 1) * W],
                                    start=(c == 0),
                                    stop=(c == DC - 1),
                                )
                            nc.vector.tensor_scalar_add(
                                kT[db][:, sc * W:(sc + 1) * W],
                                pk[:],
                                bk_t[:, db:db + 1],
                            )
                    # V: [s_block 128, d 512], bias via rank-1 ones matmul
                    vv = []
                    for sb in range(SB):
                        pv = pp.tile([P, W], f32, tag="pp")
                        for c in range(DC):
                            nc.tensor.matmul(
                                pv[:],
                                xt[c][:, sb * P:(sb + 1) * P],
                                wv[c][:],
                                start=(c == 0),
                                stop=False,
                            )
                        nc.tensor.matmul(
                            pv[:], onesr_t[:], bv_t[:], start=False, stop=True
                        )
                        tv = vp.tile([P, W], f32r, tag="vv")
                        nc.vector.tensor_copy(tv[:], pv[:])
                        vv.append(tv)

                    # ---- attention, one q-chunk (512 queries) at a time ----
                    for qc in range(QC):
                        kmax = (qc + 1) * (W // P)  # causal: k blocks needed
                        exp_tiles = []
                        for kb in range(kmax):
                            pscore = psp.tile([P, W], f32, tag="ps")
                            for dc in range(DC):
                                nc.tensor.matmul(
                                    pscore[:],
                                    kT[dc][:, kb * P:(kb + 1) * P],
                                    qT[dc][:, qc * W:(qc + 1) * W],
                                    start=(dc == 0),
                                    stop=(dc == DC - 1),
                                )
                            j = kb - qc * (W // P)
                            if j >= 0:
                                # diagonal block: add triangular causal mask
                                nc.vector.tensor_tensor(
                                    pscore[:], pscore[:], causal[j][:],
                                    op=mybir.AluOpType.add,
                                )
                            et = wep.tile([P, W], f32r, tag="we")
                            nc.scalar.activation(
                                et[:],
                                pscore[:],
                                mybir.ActivationFunctionType.Exp,
                                bias=kbias_t[:, kb:kb + 1],
                                scale=1.0,
                            )
                            exp_tiles.append(et)

                        # denominators: ones.T @ expT accumulated over k blocks
                        pden = pdp.tile([1, W], f32, tag="pd")
                        for kb in range(kmax):
                            nc.tensor.matmul(
                                pden[:],
                                ones_t[:],
                                exp_tiles[kb][:],
                                start=(kb == 0),
                                stop=(kb == kmax - 1),
                            )
                        # [1, 512] -> [4, 128] reshape DMA -> PE transpose ->
                        # [128, 4]; then scale[q] = qmask[q] / (denom[q] + eps)
                        dcp = mp.tile([1, W], f32, tag="dcp")
                        nc.scalar.copy(dcp[:], pden[:])
                        den4 = mp.tile([W // P, P], f32, tag="den4")
                        nc.sync.dma_start(out=den4[:], in_=dcp[0:1, :])
                        pdt = pdp.tile([P, W // P], f32, tag="pdt")
                        nc.tensor.transpose(
                            pdt[:], den4[:], ident[:W // P, :W // P]
                        )
                        scl = mp.tile([P, QC], f32, tag="scl")
                        nc.vector.tensor_scalar_add(scl[:], pdt[:], EPS)
                        nc.vector.reciprocal(scl[:], scl[:])
                        nc.vector.tensor_tensor(
                            scl[:],
                            scl[:],
                            qmask_t[:, qc * QC:(qc + 1) * QC],
                            op=mybir.AluOpType.mult,
                        )

                        # out_un[q,d] = sum_k expT[k,q]^T V[k,d]; then scale
                        for jq in range(W // P):
                            qb = qc * (W // P) + jq
                            pout = pop.tile([P, W], f32, tag="po")
                            for kb in range(qb + 1):
                                nc.tensor.matmul(
                                    pout[:],
                                    exp_tiles[kb][:, jq * P:(jq + 1) * P],
                                    vv[kb][:],
                                    start=(kb == 0),
                                    stop=(kb == qb),
                                )
                            ot = op_.tile([P, W], f32, tag="outs")
                            nc.vector.tensor_scalar_mul(
                                ot[:], pout[:], scl[:, jq:jq + 1]
                            )
                            nc.sync.dma_start(
                                out=out_d[seq, qb * P:(qb + 1) * P, :],
                                in_=ot[:],
                            )
    nc.finalize()
    return nc


def prep_inputs(x, Wq, bq, Wk, bk, Wv, bv, padding_mask):
    """Host-side layout prep + sharding. Returns per-core in_maps."""
    x = np.asarray(x, dtype=np.float32)
    pad = np.asarray(padding_mask).astype(bool)
    sc = 1.0 / np.sqrt(np.float32(D))
    wqT = np.ascontiguousarray(np.asarray(Wq, np.float32).T * sc)
    wkT = np.ascontiguousarray(np.asarray(Wk, np.float32).T)
    wvT = np.ascontiguousarray(np.asarray(Wv, np.float32).T)
    bq_s = (np.asarray(bq, np.float32) * sc).astype(np.float32)
    bk_a = np.asarray(bk, np.float32)
    bv_a = np.asarray(bv, np.float32).reshape(1, D)
    kbias = np.where(pad, np.float32(NEG), np.float32(0.0)).astype(np.float32)
    qmask = np.where(pad, np.float32(0.0), np.float32(1.0)).astype(np.float32)
    ones = np.ones((P, 1), dtype=np.float32)
    onesr = np.ones((1, P), dtype=np.float32)

    xT = np.ascontiguousarray(x.transpose(0, 2, 1))  # [B, D, S]

    in_maps = []
    for i in range(N_CORES):
        s0, s1 = i * BPC, (i + 1) * BPC
        in_maps.append({
            "xT": np.ascontiguousarray(xT[s0:s1]),
            "wqT": wqT, "wkT": wkT, "wvT": wvT,
            "bq": bq_s, "bk": bk_a, "bv": bv_a,
            "kbias": np.ascontiguousarray(kbias[s0:s1]),
            "qmask": np.ascontiguousarray(qmask[s0:s1]),
            "ones": ones, "onesr": onesr,
        })
    return in_maps


_NC_CACHE = {}


def get_nc(repeat: int = 1):
    if repeat not in _NC_CACHE:
        _NC_CACHE[repeat] = build_nc(repeat)
    return _NC_CACHE[repeat]


def kernel(x, Wq, bq, Wk, bk, Wv, bv, padding_mask):
    in_maps = prep_inputs(x, Wq, bq, Wk, bk, Wv, bv, padding_mask)
    nc = get_nc(1)
    r = run_bass_kernel_spmd(nc, in_maps, list(range(N_CORES)))
    out = np.concatenate([r.results[i]["out"] for i in range(N_CORES)], axis=0)
    return out.astype(np.float32)


# revision 14
# speedup vs baseline: 361.2803x; 361.2803x over previous
"""Causal self-attention (B=16, S=2048, D=512) on 8 Trainium2 NeuronCores.

Strategy: data-parallel over batch (2 sequences per core), QKV weights
replicated. Per sequence everything is computed in transposed layouts so no
on-device transposes are needed:

  host prep:  xT = x^T per sequence [D, S];  wqT = Wq^T/sqrt(D);  wkT = Wk^T;
              wvT = Wv^T;  bq' = bq/sqrt(D);  key-pad bias (0/-1e30);
              query mask (1/0) as floats.

  device (per sequence):
    QT[d,s]  = wqT^T.slices @ xT        (+bq' via DVE eviction, pre-scaled)
    KT[d,s]  = wkT^T.slices @ xT        (+bk via DVE eviction)
    V[s,d]   = xT^T.slices @ wvT        (+bv via rank-1 ones matmul)
    per q-chunk (512 queries):
      scoresT[k,q] = KT.T @ QT          (causal block-skip; diagonal blocks
                                         get a precomputed triangular -1e30
                                         tile added on DVE)
      expT = Exp(scoresT + keybias[k])  (ACT, key-pad mask via bias; no
                                         max-subtraction needed: |scores|<~40)
      denom[q] = ones.T @ expT          (PE reduction over k)
      out_un[q,d] = expT.slices^T @ V   (accumulate over k blocks <= q block)
      out = out_un * (recip(denom+eps) * qmask)[q]   (per-partition DVE scale)

softmax equivalence: exp without max-subtraction, masked entries exactly 0;
rows with a padded query are zeroed by qmask (matches reference's
post-softmax zeroing); trailing or arbitrary masks are both handled exactly
(an unpadded query q always has key q unmasked, so denom > 0).

All matmuls run in float32r (full-rate fp32 PE mode, ~1e-4 rel err).
"""

import numpy as np

import concourse.bacc as bacc
import concourse.mybir as mybir
from concourse.tile import TileContext
from concourse.bass_utils import run_bass_kernel_spmd

B, S, D = 16, 2048, 512
N_CORES = 8
BPC = B // N_CORES          # sequences per core
P = 128                     # partition dim
W = 512                     # matmul moving width (one PSUM bank of fp32)
DC = D // P                 # 4 contraction chunks of 128 over D
SB = S // P                 # 16 blocks of 128 over S (k/q/s blocks)
QC = S // W                 # 4 query chunks of 512
NEG = -1.0e30
EPS = 1.0e-30

f32 = mybir.dt.float32
f32r = mybir.dt.float32r


def build_nc(repeat: int = 1, loop: bool = False):
    nc = bacc.Bacc()

    xT_d = nc.declare_dram_parameter("xT", [BPC, D, S], f32r, isOutput=False)
    wqT_d = nc.declare_dram_parameter("wqT", [D, D], f32r, isOutput=False)
    wkT_d = nc.declare_dram_parameter("wkT", [D, D], f32r, isOutput=False)
    wvT_d = nc.declare_dram_parameter("wvT", [D, D], f32r, isOutput=False)
    bq_d = nc.declare_dram_parameter("bq", [D], f32, isOutput=False)
    bk_d = nc.declare_dram_parameter("bk", [D], f32, isOutput=False)
    bv_d = nc.declare_dram_parameter("bv", [1, D], f32r, isOutput=False)
    kbias_d = nc.declare_dram_parameter("kbias", [BPC, S], f32, isOutput=False)
    qmask_d = nc.declare_dram_parameter("qmask", [BPC, S], f32, isOutput=False)
    ones_d = nc.declare_dram_parameter("ones", [P, 1], f32r, isOutput=False)
    onesr_d = nc.declare_dram_parameter("onesr", [1, P], f32r, isOutput=False)
    out_d = nc.declare_dram_parameter("out", [BPC, S, D], f32, isOutput=True)

    with TileContext(nc) as tc:
        with (
            tc.tile_pool(name="persist", bufs=1) as pers,
            tc.tile_pool(name="xt", bufs=DC) as xp,
            tc.tile_pool(name="qt", bufs=DC) as qp,
            tc.tile_pool(name="kt", bufs=DC) as kp,
            tc.tile_pool(name="vv", bufs=SB) as vp,
            tc.tile_pool(name="we", bufs=16) as wep,
            tc.tile_pool(name="outs", bufs=3) as op_,
            tc.tile_pool(name="misc", bufs=2) as mp,
            tc.tile_pool(name="dacc", bufs=1) as dap,
            tc.tile_pool(name="pp", bufs=2, space="PSUM") as pp,
            tc.tile_pool(name="ps", bufs=2, space="PSUM") as psp,
            tc.tile_pool(name="pd", bufs=1, space="PSUM") as pdp,
            tc.tile_pool(name="po", bufs=2, space="PSUM") as pop,
        ):
            # ---- persistent setup (once) ----
            ones_t = pers.tile([P, 1], f32r, tag="ones")
            nc.sync.dma_start(out=ones_t[:], in_=ones_d[:])
            onesr_t = pers.tile([1, P], f32r, tag="onesr")
            nc.sync.dma_start(out=onesr_t[:], in_=onesr_d[:])
            bv_t = pers.tile([1, D], f32r, tag="bv")
            nc.sync.dma_start(out=bv_t[:], in_=bv_d[:])
            bq_t = pers.tile([P, DC], f32, tag="bq")
            nc.sync.dma_start(out=bq_t[:], in_=bq_d.rearrange("(n p) -> p n", p=P))
            bk_t = pers.tile([P, DC], f32, tag="bk")
            nc.sync.dma_start(out=bk_t[:], in_=bk_d.rearrange("(n p) -> p n", p=P))

            # identity for PE-mode transpose of the [4,128] denominator strip
            ident = pers.tile([P, P], f32, tag="ident")
            nc.gpsimd.memset(ident[:], 0.0)
            nc.gpsimd.affine_select(
                out=ident[:], in_=ident[:],
                compare_op=mybir.AluOpType.not_equal, fill=1.0,
                base=0, pattern=[[-1, P]], channel_multiplier=1,
            )

            # bv broadcast to all partitions via one rank-1 matmul (ones x bv)
            pbv = pdp.tile([P, W], f32, tag="pdt")
            nc.tensor.matmul(pbv[:], onesr_t[:], bv_t[:], start=True, stop=True)
            bvb_t = pers.tile([P, W], f32, tag="bvb")
            nc.vector.tensor_copy(bvb_t[:], pbv[:])

            # causal tiles for the 4 diagonal sub-blocks of a [k=128, q=512]
            # scoresT tile: mask (set NEG) where 128*j + k_local > q_local
            causal = []
            for j in range(W // P):
                ct = pers.tile([P, W], f32, tag=f"causal{j}")
                nc.gpsimd.memset(ct[:], 0.0)
                nc.gpsimd.affine_select(
                    out=ct[:],
                    in_=ct[:],
                    compare_op=mybir.AluOpType.is_ge,
                    fill=NEG,
                    base=-P * j,
                    pattern=[[1, W]],
                    channel_multiplier=-1,
                )
                causal.append(ct)

            import contextlib
            rep_ctx = (
                tc.For_i(0, repeat, 1) if loop else contextlib.nullcontext(0)
            )
            with rep_ctx:
              for _rep in range(1 if loop else repeat):
                for seq in range(BPC):
                    # ---- per-sequence masks ----
                    kbias_t = mp.tile([P, SB], f32, tag="kbias")
                    nc.sync.dma_start(
                        out=kbias_t[:],
                        in_=kbias_d[seq].rearrange("(n p) -> p n", p=P),
                    )
                    qmask_t = mp.tile([P, SB], f32, tag="qmask")
                    nc.sync.dma_start(
                        out=qmask_t[:],
                        in_=qmask_d[seq].rearrange("(n p) -> p n", p=P),
                    )

                    # ---- load xT (4 tiles of [128, S]) ----
                    xt = []
                    for c in range(DC):
                        t = xp.tile([P, S], f32r, tag="xt")
                        nc.sync.dma_start(
                            out=t[:], in_=xT_d[seq, c * P:(c + 1) * P, :]
                        )
                        xt.append(t)

                    # ---- load weights (shared slots with exp tiles) ----
                    wq, wk, wv = [], [], []
                    for c in range(DC):
                        t = wep.tile([P, W], f32r, tag="we")
                        nc.sync.dma_start(out=t[:], in_=wqT_d[c * P:(c + 1) * P, :])
                        wq.append(t)
                    for c in range(DC):
                        t = wep.tile([P, W], f32r, tag="we")
                        nc.sync.dma_start(out=t[:], in_=wkT_d[c * P:(c + 1) * P, :])
                        wk.append(t)
                    for c in range(DC):
                        t = wep.tile([P, W], f32r, tag="we")
                        nc.sync.dma_start(out=t[:], in_=wvT_d[c * P:(c + 1) * P, :])
                        wv.append(t)

                    # ---- projections ----
                    # QT/KT: [d_block 128, s] accumulated over c, bias via DVE
                    qT, kT = [], []
                    for db in range(DC):
                        tq = qp.tile([P, S], f32r, tag="qt")
                        tk = kp.tile([P, S], f32r, tag="kt")
                        qT.append(tq)
                        kT.append(tk)
                    for db in range(DC):
                        for sc in range(QC):
                            pq = pp.tile([P, W], f32, tag="pp")
                            for c in range(DC):
                                nc.tensor.matmul(
                                    pq[:],
                                    wq[c][:, db * P:(db + 1) * P],
                                    xt[c][:, sc * W:(sc + 1) * W],
                                    start=(c == 0),
                                    stop=(c == DC - 1),
                                )
                            nc.vector.tensor_scalar_add(
                                qT[db][:, sc * W:(sc + 1) * W],
                                pq[:],
                                bq_t[:, db:db + 1],
                            )
                            pk = pp.tile([P, W], f32, tag="pp")
                            for c in range(DC):
                                nc.tensor.matmul(
                                    pk[:],
                                    wk[c][:, db * P:(db + 1) * P],
                                    xt[c][:, sc * W:(sc + 1) * W],
                                    start=(c == 0),
                                    stop=(c == DC - 1),
                                )
                            nc.vector.tensor_scalar_add(
                                kT[db][:, sc * W:(sc + 1) * W],
                                pk[:],
                                bk_t[:, db:db + 1],
                            )
                    # V: [s_block 128, d 512], bias added during DVE eviction
                    vv = []
                    for sb in range(SB):
                        pv = pp.tile([P, W], f32, tag="pp")
                        for c in range(DC):
                            nc.tensor.matmul(
                                pv[:],
                                xt[c][:, sb * P:(sb + 1) * P],
                                wv[c][:],
                                start=(c == 0),
                                stop=(c == DC - 1),
                            )
                        tv = vp.tile([P, W], f32r, tag="vv")
                        nc.vector.tensor_add(tv[:], pv[:], bvb_t[:])
                        vv.append(tv)

                    # ---- attention, one q-chunk (512 queries) at a time ----
                    for qc in range(QC):
                        kmax = (qc + 1) * (W // P)  # causal: k blocks needed
                        exp_tiles = []
                        for kb in range(kmax):
                            pscore = psp.tile([P, W], f32, tag="ps")
                            for dc in range(DC):
                                nc.tensor.matmul(
                                    pscore[:],
                                    kT[dc][:, kb * P:(kb + 1) * P],
                                    qT[dc][:, qc * W:(qc + 1) * W],
                                    start=(dc == 0),
                                    stop=(dc == DC - 1),
                                )
                            j = kb - qc * (W // P)
                            if j >= 0:
                                # diagonal block: add triangular causal mask
                                nc.vector.tensor_tensor(
                                    pscore[:], pscore[:], causal[j][:],
                                    op=mybir.AluOpType.add,
                                )
                            et = wep.tile([P, W], f32r, tag="we")
                            nc.scalar.activation(
                                et[:],
                                pscore[:],
                                mybir.ActivationFunctionType.Exp,
                                bias=kbias_t[:, kb:kb + 1],
                                scale=1.0,
                            )
                            exp_tiles.append(et)

                        # denominators: DVE-accumulate exp tiles over k blocks,
                        # then a single ones.T @ acc matmul per q-chunk
                        dacc = dap.tile([P, W], f32r, tag="dacc")
                        nc.vector.tensor_add(
                            dacc[:], exp_tiles[0][:], exp_tiles[1][:]
                        )
                        for kb in range(2, kmax):
                            nc.vector.tensor_add(
                                dacc[:], dacc[:], exp_tiles[kb][:]
                            )
                        pden = pdp.tile([1, W], f32, tag="pd")
                        nc.tensor.matmul(
                            pden[:], ones_t[:], dacc[:], start=True, stop=True
                        )
                        # [1, 512] -> [4, 128] reshape DMA -> PE transpose ->
                        # [128, 4]; then scale[q] = qmask[q] / (denom[q] + eps)
                        dcp = mp.tile([1, W], f32, tag="dcp")
                        nc.scalar.copy(dcp[:], pden[:])
                        den4 = mp.tile([W // P, P], f32, tag="den4")
                        nc.sync.dma_start(out=den4[:], in_=dcp[0:1, :])
                        pdt = pdp.tile([P, W // P], f32, tag="pdt")
                        nc.tensor.transpose(
                            pdt[:], den4[:], ident[:W // P, :W // P]
                        )
                        scl = mp.tile([P, QC], f32, tag="scl")
                        nc.vector.tensor_scalar_add(scl[:], pdt[:], EPS)
                        nc.vector.reciprocal(scl[:], scl[:])
                        nc.vector.tensor_tensor(
                            scl[:],
                            scl[:],
                            qmask_t[:, qc * QC:(qc + 1) * QC],
                            op=mybir.AluOpType.mult,
                        )

                        # out_un[q,d] = sum_k expT[k,q]^T V[k,d]; then scale
                        for jq in range(W // P):
                            qb = qc * (W // P) + jq
                            pout = pop.tile([P, W], f32, tag="po")
                            for kb in range(qb + 1):
                                nc.tensor.matmul(
                                    pout[:],
                                    exp_tiles[kb][:, jq * P:(jq + 1) * P],
                                    vv[kb][:],
                                    start=(kb == 0),
                                    stop=(kb == qb),
                                )
                            ot = op_.tile([P, W], f32, tag="outs")
                            nc.vector.tensor_scalar_mul(
                                ot[:], pout[:], scl[:, jq:jq + 1]
                            )
                            nc.sync.dma_start(
                                out=out_d[seq, qb * P:(qb + 1) * P, :],
                                in_=ot[:],
                            )
    nc.finalize()
    return nc


def prep_inputs(x, Wq, bq, Wk, bk, Wv, bv, padding_mask):
    """Host-side layout prep + sharding. Returns per-core in_maps."""
    x = np.asarray(x, dtype=np.float32)
    pad = np.asarray(padding_mask).astype(bool)
    sc = 1.0 / np.sqrt(np.float32(D))
    wqT = np.ascontiguousarray(np.asarray(Wq, np.float32).T * sc)
    wkT = np.ascontiguousarray(np.asarray(Wk, np.float32).T)
    wvT = np.ascontiguousarray(np.asarray(Wv, np.float32).T)
    bq_s = (np.asarray(bq, np.float32) * sc).astype(np.float32)
    bk_a = np.asarray(bk, np.float32)
    bv_a = np.asarray(bv, np.float32).reshape(1, D)
    kbias = np.where(pad, np.float32(NEG), np.float32(0.0)).astype(np.float32)
    qmask = np.where(pad, np.float32(0.0), np.float32(1.0)).astype(np.float32)
    ones = np.ones((P, 1), dtype=np.float32)
    onesr = np.ones((1, P), dtype=np.float32)

    xT = np.ascontiguousarray(x.transpose(0, 2, 1))  # [B, D, S]

    in_maps = []
    for i in range(N_CORES):
        s0, s1 = i * BPC, (i + 1) * BPC
        in_maps.append({
            "xT": np.ascontiguousarray(xT[s0:s1]),
            "wqT": wqT, "wkT": wkT, "wvT": wvT,
            "bq": bq_s, "bk": bk_a, "bv": bv_a,
            "kbias": np.ascontiguousarray(kbias[s0:s1]),
            "qmask": np.ascontiguousarray(qmask[s0:s1]),
            "ones": ones, "onesr": onesr,
        })
    return in_maps


_NC_CACHE = {}


def get_nc(repeat: int = 1, loop: bool = False):
    key = (repeat, loop)
    if key not in _NC_CACHE:
        _NC_CACHE[key] = build_nc(repeat, loop)
    return _NC_CACHE[key]


def kernel(x, Wq, bq, Wk, bk, Wv, bv, padding_mask):
    in_maps = prep_inputs(x, Wq, bq, Wk, bk, Wv, bv, padding_mask)
    nc = get_nc(1)
    r = run_bass_kernel_spmd(nc, in_maps, list(range(N_CORES)))
    out = np.concatenate([r.results[i]["out"] for i in range(N_CORES)], axis=0)
    return out.astype(np.float32)


# revision 25
# speedup vs baseline: 495.6517x; 1.3719x over previous
"""Causal self-attention (B=16, S=2048, D=512) on 8 Trainium2 NeuronCores.

Strategy: data-parallel over batch (2 sequences per core), QKV weights
replicated. Per sequence everything is computed in transposed layouts so no
on-device transposes are needed:

  host prep:  xT = x^T per sequence [D, S];  wqT = Wq^T/sqrt(D);  wkT = Wk^T;
              wvT = Wv^T;  bq' = bq/sqrt(D);  key-pad bias (0/-1e30);
              query mask (1/0) as floats.

  device (per sequence):
    QT[d,s]  = wqT^T.slices @ xT        (+bq' via DVE eviction, pre-scaled)
    KT[d,s]  = wkT^T.slices @ xT        (+bk via DVE eviction)
    V[s,d]   = xT^T.slices @ wvT        (+bv via rank-1 ones matmul)
    per q-chunk (512 queries):
      scoresT[k,q] = KT.T @ QT          (causal block-skip; diagonal blocks
                                         get a precomputed triangular -1e30
                                         tile added on DVE)
      expT = Exp(scoresT + keybias[k])  (ACT, key-pad mask via bias; no
                                         max-subtraction needed: |scores|<~40)
      denom[q] = ones.T @ expT          (PE reduction over k)
      out_un[q,d] = expT.slices^T @ V   (accumulate over k blocks <= q block)
      out = out_un * (recip(denom+eps) * qmask)[q]   (per-partition DVE scale)

softmax equivalence: exp without max-subtraction, masked entries exactly 0;
rows with a padded query are zeroed by qmask (matches reference's
post-softmax zeroing); trailing or arbitrary masks are both handled exactly
(an unpadded query q always has key q unmasked, so denom > 0).

All matmuls run in float32r (full-rate fp32 PE mode, ~1e-4 rel err).
"""

import numpy as np

import concourse.bacc as bacc
import concourse.mybir as mybir
from concourse.tile import TileContext
from concourse.bass_utils import run_bass_kernel_spmd

B, S, D = 16, 2048, 512
N_CORES = 8
BPC = B // N_CORES          # sequences per core
P = 128                     # partition dim
W = 512                     # matmul moving width (one PSUM bank of fp32)
DC = D // P                 # 4 contraction chunks of 128 over D
SB = S // P                 # 16 blocks of 128 over S (k/q/s blocks)
QC = S // W                 # 4 query chunks of 512
NEG = -1.0e30
EPS = 1.0e-30

f32 = mybir.dt.float32
f32r = mybir.dt.float32r


def build_nc(repeat: int = 1, loop: bool = False, slot_caps=(SB, SB)):
    """slot_caps[s] = number of 128-blocks of valid (non-padded) positions for
    sequence slot s on every core (program-wide). Blocks beyond the cap hold
    only padded positions: their keys contribute exactly 0 (key bias) and
    their query rows are exactly 0 in the reference (query mask), so skipping
    them and zero-filling the output rows is exact for any mask."""
    nc = bacc.Bacc()

    xT_d = nc.declare_dram_parameter("xT", [BPC, D, S], f32r, isOutput=False)
    wqT_d = nc.declare_dram_parameter("wqT", [D, D], f32r, isOutput=False)
    wkT_d = nc.declare_dram_parameter("wkT", [D, D], f32r, isOutput=False)
    wvT_d = nc.declare_dram_parameter("wvT", [D, D], f32r, isOutput=False)
    bq_d = nc.declare_dram_parameter("bq", [D], f32, isOutput=False)
    bk_d = nc.declare_dram_parameter("bk", [D], f32, isOutput=False)
    bv_d = nc.declare_dram_parameter("bv", [1, D], f32r, isOutput=False)
    kbias_d = nc.declare_dram_parameter("kbias", [BPC, S], f32, isOutput=False)
    qmask_d = nc.declare_dram_parameter("qmask", [BPC, S], f32, isOutput=False)
    ones_d = nc.declare_dram_parameter("ones", [P, 1], f32r, isOutput=False)
    onesr_d = nc.declare_dram_parameter("onesr", [1, P], f32r, isOutput=False)
    out_d = nc.declare_dram_parameter("out", [BPC, S, D], f32, isOutput=True)

    with TileContext(nc) as tc:
        with (
            tc.tile_pool(name="persist", bufs=1) as pers,
            tc.tile_pool(name="xt", bufs=DC) as xp,
            tc.tile_pool(name="qt", bufs=DC) as qp,
            tc.tile_pool(name="kt", bufs=DC) as kp,
            tc.tile_pool(name="vv", bufs=SB) as vp,
            tc.tile_pool(name="we", bufs=16) as wep,
            tc.tile_pool(name="outs", bufs=3) as op_,
            tc.tile_pool(name="misc", bufs=2) as mp,
            tc.tile_pool(name="dacc", bufs=1) as dap,
            tc.tile_pool(name="pp", bufs=2, space="PSUM") as pp,
            tc.tile_pool(name="ps", bufs=2, space="PSUM") as psp,
            tc.tile_pool(name="pd", bufs=1, space="PSUM") as pdp,
            tc.tile_pool(name="po", bufs=2, space="PSUM") as pop,
        ):
            # ---- persistent setup (once) ----
            ones_t = pers.tile([P, 1], f32r, tag="ones")
            nc.sync.dma_start(out=ones_t[:], in_=ones_d[:])
            onesr_t = pers.tile([1, P], f32r, tag="onesr")
            nc.sync.dma_start(out=onesr_t[:], in_=onesr_d[:])
            bv_t = pers.tile([1, D], f32r, tag="bv")
            nc.sync.dma_start(out=bv_t[:], in_=bv_d[:])
            bq_t = pers.tile([P, DC], f32, tag="bq")
            nc.sync.dma_start(out=bq_t[:], in_=bq_d.rearrange("(n p) -> p n", p=P))
            bk_t = pers.tile([P, DC], f32, tag="bk")
            nc.sync.dma_start(out=bk_t[:], in_=bk_d.rearrange("(n p) -> p n", p=P))

            # identity for PE-mode transpose of the [4,128] denominator strip
            ident = pers.tile([P, P], f32, tag="ident")
            nc.gpsimd.memset(ident[:], 0.0)
            nc.gpsimd.affine_select(
                out=ident[:], in_=ident[:],
                compare_op=mybir.AluOpType.not_equal, fill=1.0,
                base=0, pattern=[[-1, P]], channel_multiplier=1,
            )

            # bv broadcast to all partitions via one rank-1 matmul (ones x bv)
            pbv = pdp.tile([P, W], f32, tag="pdt")
            nc.tensor.matmul(pbv[:], onesr_t[:], bv_t[:], start=True, stop=True)
            bvb_t = pers.tile([P, W], f32, tag="bvb")
            nc.vector.tensor_copy(bvb_t[:], pbv[:])

            # zero tile for output rows beyond a slot's valid-block cap
            zt = pers.tile([P, W], f32, tag="zt")
            nc.gpsimd.memset(zt[:], 0.0)

            # causal tiles for the 4 diagonal sub-blocks of a [k=128, q=512]
            # scoresT tile: mask (set NEG) where 128*j + k_local > q_local
            causal = []
            for j in range(W // P):
                ct = pers.tile([P, W], f32, tag=f"causal{j}")
                nc.gpsimd.memset(ct[:], 0.0)
                nc.gpsimd.affine_select(
                    out=ct[:],
                    in_=ct[:],
                    compare_op=mybir.AluOpType.is_ge,
                    fill=NEG,
                    base=-P * j,
                    pattern=[[1, W]],
                    channel_multiplier=-1,
                )
                causal.append(ct)

            import contextlib
            rep_ctx = (
                tc.For_i(0, repeat, 1) if loop else contextlib.nullcontext(0)
            )
            with rep_ctx:
              for _rep in range(1 if loop else repeat):
                for seq in range(BPC):
                    KB = slot_caps[seq]       # valid 128-blocks this slot
                    SCcap = -(-KB // (W // P))  # q-chunks covering them
                    # ---- per-sequence masks ----
                    kbias_t = mp.tile([P, SB], f32, tag="kbias")
                    nc.sync.dma_start(
                        out=kbias_t[:],
                        in_=kbias_d[seq].rearrange("(n p) -> p n", p=P),
                    )
                    qmask_t = mp.tile([P, SB], f32, tag="qmask")
                    nc.sync.dma_start(
                        out=qmask_t[:],
                        in_=qmask_d[seq].rearrange("(n p) -> p n", p=P),
                    )

                    # ---- load xT (4 tiles of [128, S]) ----
                    xt = []
                    for c in range(DC):
                        t = xp.tile([P, S], f32r, tag="xt")
                        nc.sync.dma_start(
                            out=t[:], in_=xT_d[seq, c * P:(c + 1) * P, :]
                        )
                        xt.append(t)

                    # ---- load weights (shared slots with exp tiles) ----
                    wq, wk, wv = [], [], []
                    for c in range(DC):
                        t = wep.tile([P, W], f32r, tag="we")
                        nc.sync.dma_start(out=t[:], in_=wqT_d[c * P:(c + 1) * P, :])
                        wq.append(t)
                    for c in range(DC):
                        t = wep.tile([P, W], f32r, tag="we")
                        nc.sync.dma_start(out=t[:], in_=wkT_d[c * P:(c + 1) * P, :])
                        wk.append(t)
                    for c in range(DC):
                        t = wep.tile([P, W], f32r, tag="we")
                        nc.sync.dma_start(out=t[:], in_=wvT_d[c * P:(c + 1) * P, :])
                        wv.append(t)

                    # ---- projections ----
                    # QT/KT: [d_block 128, s] accumulated over c, bias via DVE
                    qT, kT = [], []
                    for db in range(DC):
                        tq = qp.tile([P, S], f32r, tag="qt")
                        tk = kp.tile([P, S], f32r, tag="kt")
                        qT.append(tq)
                        kT.append(tk)
                    for db in range(DC):
                        for sc in range(SCcap):
                            pq = pp.tile([P, W], f32, tag="pp")
                            for c in range(DC):
                                nc.tensor.matmul(
                                    pq[:],
                                    wq[c][:, db * P:(db + 1) * P],
                                    xt[c][:, sc * W:(sc + 1) * W],
                                    start=(c == 0),
                                    stop=(c == DC - 1),
                                )
                            nc.vector.tensor_scalar_add(
                                qT[db][:, sc * W:(sc + 1) * W],
                                pq[:],
                                bq_t[:, db:db + 1],
                            )
                            pk = pp.tile([P, W], f32, tag="pp")
                            for c in range(DC):
                                nc.tensor.matmul(
                                    pk[:],
                                    wk[c][:, db * P:(db + 1) * P],
                                    xt[c][:, sc * W:(sc + 1) * W],
                                    start=(c == 0),
                                    stop=(c == DC - 1),
                                )
                            nc.vector.tensor_scalar_add(
                                kT[db][:, sc * W:(sc + 1) * W],
                                pk[:],
                                bk_t[:, db:db + 1],
                            )
                    # V: [s_block 128, d 512], bias added during DVE eviction
                    vv = []
                    for sb in range(KB):
                        pv = pp.tile([P, W], f32, tag="pp")
                        for c in range(DC):
                            nc.tensor.matmul(
                                pv[:],
                                xt[c][:, sb * P:(sb + 1) * P],
                                wv[c][:],
                                start=(c == 0),
                                stop=(c == DC - 1),
                            )
                        tv = vp.tile([P, W], f32r, tag="vv")
                        nc.vector.tensor_add(tv[:], pv[:], bvb_t[:])
                        vv.append(tv)

                    # ---- attention, one q-chunk (512 queries) at a time ----
                    for qc in range(SCcap):
                        kmax = min((qc + 1) * (W // P), KB)  # causal + cap
                        exp_tiles = []
                        for kb in range(kmax):
                            pscore = psp.tile([P, W], f32, tag="ps")
                            for dc in range(DC):
                                nc.tensor.matmul(
                                    pscore[:],
                                    kT[dc][:, kb * P:(kb + 1) * P],
                                    qT[dc][:, qc * W:(qc + 1) * W],
                                    start=(dc == 0),
                                    stop=(dc == DC - 1),
                                )
                            j = kb - qc * (W // P)
                            if j >= 0:
                                # diagonal block: add triangular causal mask
                                nc.vector.tensor_tensor(
                                    pscore[:], pscore[:], causal[j][:],
                                    op=mybir.AluOpType.add,
                                )
                            et = wep.tile([P, W], f32r, tag="we")
                            nc.scalar.activation(
                                et[:],
                                pscore[:],
                                mybir.ActivationFunctionType.Exp,
                                bias=kbias_t[:, kb:kb + 1],
                                scale=1.0,
                            )
                            exp_tiles.append(et)

                        # denominators: DVE-accumulate exp tiles over k blocks,
                        # then a single ones.T @ acc matmul per q-chunk
                        dacc = dap.tile([P, W], f32r, tag="dacc")
                        if kmax == 1:
                            nc.vector.tensor_copy(dacc[:], exp_tiles[0][:])
                        else:
                            nc.vector.tensor_add(
                                dacc[:], exp_tiles[0][:], exp_tiles[1][:]
                            )
                            for kb in range(2, kmax):
                                nc.vector.tensor_add(
                                    dacc[:], dacc[:], exp_tiles[kb][:]
                                )
                        pden = pdp.tile([1, W], f32, tag="pd")
                        nc.tensor.matmul(
                            pden[:], ones_t[:], dacc[:], start=True, stop=True
                        )
                        # [1, 512] -> [4, 128] reshape DMA -> PE transpose ->
                        # [128, 4]; then scale[q] = qmask[q] / (denom[q] + eps)
                        dcp = mp.tile([1, W], f32, tag="dcp")
                        nc.scalar.copy(dcp[:], pden[:])
                        den4 = mp.tile([W // P, P], f32, tag="den4")
                        nc.sync.dma_start(out=den4[:], in_=dcp[0:1, :])
                        pdt = pdp.tile([P, W // P], f32, tag="pdt")
                        nc.tensor.transpose(
                            pdt[:], den4[:], ident[:W // P, :W // P]
                        )
                        scl = mp.tile([P, QC], f32, tag="scl")
                        nc.vector.tensor_scalar_add(scl[:], pdt[:], EPS)
                        nc.vector.reciprocal(scl[:], scl[:])
                        nc.vector.tensor_tensor(
                            scl[:],
                            scl[:],
                            qmask_t[:, qc * QC:(qc + 1) * QC],
                            op=mybir.AluOpType.mult,
                        )

                        # out_un[q,d] = sum_k expT[k,q]^T V[k,d]; then scale
                        for jq in range(W // P):
                            qb = qc * (W // P) + jq
                            if qb >= KB:
                                continue  # all-padded query rows: zero-filled
                            pout = pop.tile([P, W], f32, tag="po")
                            for kb in range(min(qb + 1, KB)):
                                nc.tensor.matmul(
                                    pout[:],
                                    exp_tiles[kb][:, jq * P:(jq + 1) * P],
                                    vv[kb][:],
                                    start=(kb == 0),
                                    stop=(kb == qb),
                                )
                            ot = op_.tile([P, W], f32, tag="outs")
                            nc.vector.tensor_scalar_mul(
                                ot[:], pout[:], scl[:, jq:jq + 1]
                            )
                            nc.sync.dma_start(
                                out=out_d[seq, qb * P:(qb + 1) * P, :],
                                in_=ot[:],
                            )

                    # rows in blocks >= KB are entirely padded queries: zero
                    for qb in range(KB, SB):
                        nc.sync.dma_start(
                            out=out_d[seq, qb * P:(qb + 1) * P, :],
                            in_=zt[:],
                        )
    nc.finalize()
    return nc


def prep_inputs(x, Wq, bq, Wk, bk, Wv, bv, padding_mask):
    """Host-side layout prep + sharding. Returns per-core in_maps."""
    x = np.asarray(x, dtype=np.float32)
    pad = np.asarray(padding_mask).astype(bool)
    sc = 1.0 / np.sqrt(np.float32(D))
    wqT = np.ascontiguousarray(np.asarray(Wq, np.float32).T * sc)
    wkT = np.ascontiguousarray(np.asarray(Wk, np.float32).T)
    wvT = np.ascontiguousarray(np.asarray(Wv, np.float32).T)
    bq_s = (np.asarray(bq, np.float32) * sc).astype(np.float32)
    bk_a = np.asarray(bk, np.float32)
    bv_a = np.asarray(bv, np.float32).reshape(1, D)
    kbias = np.where(pad, np.float32(NEG), np.float32(0.0)).astype(np.float32)
    qmask = np.where(pad, np.float32(0.0), np.float32(1.0)).astype(np.float32)
    ones = np.ones((P, 1), dtype=np.float32)
    onesr = np.ones((1, P), dtype=np.float32)

    xT = np.ascontiguousarray(x.transpose(0, 2, 1))  # [B, D, S]

    # per-seq valid-block cap from the actual mask (exact for any mask):
    # blocks after the last non-padded position hold only padded positions
    valid = ~pad
    caps = np.zeros(B, dtype=np.int64)
    for b in range(B):
        idx = np.nonzero(valid[b])[0]
        caps[b] = 0 if idx.size == 0 else int(np.ceil((idx[-1] + 1) / P))
    order = np.argsort(-caps, kind="stable")  # descending cap
    # core i runs (slot0 = order[2*N_CORES-1-i] short, slot1 = order[i] long)
    perm = []
    for i in range(N_CORES):
        perm.extend([int(order[B - 1 - i]), int(order[i])])
    slot_caps = (int(caps[order[N_CORES]]), int(caps[order[0]]))

    in_maps = []
    for i in range(N_CORES):
        sel = [perm[2 * i], perm[2 * i + 1]]
        in_maps.append({
            "xT": np.ascontiguousarray(xT[sel]),
            "wqT": wqT, "wkT": wkT, "wvT": wvT,
            "bq": bq_s, "bk": bk_a, "bv": bv_a,
            "kbias": np.ascontiguousarray(kbias[sel]),
            "qmask": np.ascontiguousarray(qmask[sel]),
            "ones": ones, "onesr": onesr,
        })
    return in_maps, perm, slot_caps


_NC_CACHE = {}


def get_nc(repeat: int = 1, loop: bool = False, slot_caps=(SB, SB)):
    key = (repeat, loop, slot_caps)
    if key not in _NC_CACHE:
        _NC_CACHE[key] = build_nc(repeat, loop, slot_caps)
    return _NC_CACHE[key]


def kernel(x, Wq, bq, Wk, bk, Wv, bv, padding_mask):
    in_maps, perm, slot_caps = prep_inputs(
        x, Wq, bq, Wk, bk, Wv, bv, padding_mask)
    nc = get_nc(1, slot_caps=slot_caps)
    r = run_bass_kernel_spmd(nc, in_maps, list(range(N_CORES)))
    out = np.empty((B, S, D), dtype=np.float32)
    for j, orig in enumerate(perm):
        out[orig] = r.results[j // BPC]["out"][j % BPC]
    return out
